# revision 1
# baseline (speedup 1.0000x reference)
"""Causal multi-head attention kernel for Trainium2 (8 NeuronCores).

Problem: B=2, H=16, S=2048, D=64 causal attention (softmax over last axis).
Sharding: 32 (batch, head) pairs split 4-per-core across 8 cores; each core
computes its heads independently (no collectives).

Per-core algorithm (all in the "S-transposed" layout so no transposes of the
probability matrix are ever needed):
  - Host pre-packs, per core:
      qt  [2, 128, 2048] bf16 : two head-PAIRS; partitions 0-63 = head 2p's
                                Q^T (d-major), partitions 64-127 = head 2p+1's
      kt  [2, 128, 2048] bf16 : same for K
      vg  [4, 128, 16, 65] bf16: per head, V tiles [128, 65] with a ones
                                column appended (col 64) -> PV matmul also
                                produces the softmax row-sum for free
      tri [128, 128] bf16     : tri[k, c] = 1 if c >= k else 0 (intra-block
                                causal keep-mask in S^T coords)
  - For each head, for each k-block kb (128 keys):
      S^T strip [k=128, q=kb*128..2047] = K_blk^T.T @ Q^T  (PE, bf16)
      P^T = exp(S^T / 8)               (ACT, PSUM->SBUF, bf16 out)
      diagonal 128x128 block masked via elementwise tri multiply (DVE)
      O accumulation: for each 128-row q block, acc[q] += P^T_chunk.T @ V_blk
        (PE, accumulating in PSUM; 16 accumulators packed 7/7/2 per bank,
         one has_written "zero region" start per bank)
  - Normalize: O[q, :64] * (1 / O[q, 64]) (DVE reciprocal + per-partition
    scalar multiply), DMA out as f32.

kernel(Q, K, V, mask) takes the full unsharded fp32 inputs and returns the
full [2, 16, 2048, 64] fp32 output.
"""

import sys

if "/opt/trn_rl_repo" not in sys.path:
    sys.path.insert(0, "/opt/trn_rl_repo")

import numpy as np
import ml_dtypes

B, H, S, D = 2, 16, 2048, 64
N_CORES = 8
HEADS_PER_CORE = (B * H) // N_CORES  # 4
KB = S // 128  # 16 k-blocks per head
QS = S // 128  # 16 q-subblocks per head

_BF16 = ml_dtypes.bfloat16

# accumulator bank packing: q_subs 0-6 -> bank A, 7-13 -> bank B, 14-15 -> C
_BANK_FIRST = (0, 7, 14)   # first q_sub written in each accumulator bank
_BANK_LAST = (6, 13, 15)   # last q_sub written in each accumulator bank

_built = {}
STRIP_W = 1024
MODE = "full"  # full | qk_only | qk_exp (timing ablations)
ST_BUFS = 2  # PSUM strip-tile slots  # S^T strip tile width (PSUM free elems)


def _emit(tc, nc, mybir, qt, kt, vg, tri, o, causal, reps=1):
    from contextlib import ExitStack

    f32 = mybir.dt.float32
    bf = mybir.dt.bfloat16
    Exp = mybir.ActivationFunctionType.Exp

    with ExitStack() as ctx:
        const = ctx.enter_context(tc.tile_pool(name="const", bufs=1))
        qk = ctx.enter_context(tc.tile_pool(name="qk", bufs=2))
        vpool = ctx.enter_context(tc.tile_pool(name="vp", bufs=2))
        ptp = ctx.enter_context(tc.tile_pool(name="ptp", bufs=4))
        outp = ctx.enter_context(tc.tile_pool(name="outp", bufs=4))
        small = ctx.enter_context(tc.tile_pool(name="small", bufs=4))
        stp = ctx.enter_context(tc.tile_pool(name="stp", bufs=ST_BUFS, space="PSUM"))
        accp = ctx.enter_context(tc.tile_pool(name="accp", bufs=1, space="PSUM"))

        tri_t = const.tile([128, 128], bf, name="tri_t")
        nc.sync.dma_start(tri_t[:, :], tri[:, :])

        # dummy exp issued first: walrus places the ~2.7us ACT table load
        # before the first ACTIVATE in the stream, so doing one on a tiny
        # constant tile overlaps the table load with the input DMAs instead
        # of serializing it before the first real exp
        warm = const.tile([128, 1], f32, name="warm")
        nc.vector.memset(warm[:, :], 0.0)
        nc.scalar.activation(warm[:, :], warm[:, :], Exp)

        from contextlib import nullcontext
        with (tc.For_i(0, reps, 1) if reps > 1 else nullcontext()):
          rep = 0  # body emitted once; hardware loop repeats it
          for p in range(HEADS_PER_CORE // 2):
              # qt via SP queue, kt via DVE queue so the two big loads overlap;
              # chunk them so the first QK matmul can start early.
              qt_t = qk.tile([128, S], bf, tag="qt", name=f"qt_{rep}_{p}")
              kt_t = qk.tile([128, S], bf, tag="kt", name=f"kt_{rep}_{p}")
              # kt on the gpsimd SWDGE queue, qt on the SP HWDGE queue so they
              # load in parallel; the leading chunks unblock the first QK early.
              nc.gpsimd.dma_start(kt_t[:, :128], kt[p][:, :128])
              nc.sync.dma_start(qt_t[:, :512], qt[p][:, :512])
              nc.sync.dma_start(qt_t[:, 512:1024], qt[p][:, 512:1024])
              nc.gpsimd.dma_start(kt_t[:, 128:], kt[p][:, 128:])
              nc.sync.dma_start(qt_t[:, 1024:], qt[p][:, 1024:])
              for s2 in range(2):
                  h = 2 * p + s2
                  po = 64 * s2  # partition offset of this head's d-dim
                  vg_t = vpool.tile([128, KB, 65], bf, tag="vg", name=f"vg_{rep}_{h}")
                  nc.gpsimd.dma_start(vg_t[:, :, :], vg[h])

                  accA = accp.tile([128, 7, 65], f32, tag="accA", name=f"accA_{rep}_{h}")
                  accB = accp.tile([128, 7, 65], f32, tag="accB", name=f"accB_{rep}_{h}")
                  accC = accp.tile([128, 2, 65], f32, tag="accC", name=f"accC_{rep}_{h}")

                  def acc(i):
                      if i < 7:
                          return accA[:, i, :]
                      if i < 14:
                          return accB[:, i - 7, :]
                      return accC[:, i - 14, :]

                  def strip_halves(kb):
                      q0 = 128 * kb if causal else 0
                      cols = S - q0
                      pieces = []
                      hs = 0
                      while hs < cols:
                          pieces.append((q0, hs, min(1024, cols - hs), "A"))
                          hs += 1024
                      return pieces

                  def emit_qk(kb):
                      """QK matmuls for strip kb; returns the st tiles."""
                      sts = []
                      for q0, hs, hw, tg in strip_halves(kb):
                          st = stp.tile([128, 1024], f32, tag="st",
                                        name=f"st_{rep}_{h}_{kb}_{hs}")
                          for c0 in range(0, hw, 512):
                              cw = min(512, hw - c0)
                              nc.tensor.matmul(
                                  st[:, c0:c0 + cw],
                                  lhsT=kt_t[po:po + 64, kb * 128:(kb + 1) * 128],
                                  rhs=qt_t[po:po + 64,
                                           q0 + hs + c0:q0 + hs + c0 + cw],
                                  start=True, stop=True,
                              )
                          sts.append(st)
                      return sts

                  def normalize_bank(qs_lo, qs_hi):
                      """Normalize q_subs [qs_lo, qs_hi) and write out in one
                      batched DMA (rows r of out tile j map to q = qs*128+r)."""
                      n = qs_hi - qs_lo
                      ot = outp.tile([128, n, 64], f32, tag=f"ot{qs_lo}",
                                     name=f"ot_{rep}_{h}_{qs_lo}")
                      for j in range(n):
                          a = acc(qs_lo + j)
                          rs = small.tile([128, 1], f32, tag="rs",
                                          name=f"rs_{rep}_{h}_{qs_lo + j}")
                          nc.vector.reciprocal(rs[:, :], a[:, 64:65])
                          nc.vector.tensor_scalar_mul(ot[:, j, :], a[:, :64],
                                                      rs[:, :])
                      dst = o[h, qs_lo * 128:qs_hi * 128, :].rearrange(
                          "(j r) c -> r j c", r=128)
                      nc.sync.dma_start(dst, ot[:, :, :])

                  sts = emit_qk(0)
                  for kb in range(KB):
                      # exp of strip kb
                      pts = []
                      for (q0, hs, hw, tg), st in zip(strip_halves(kb), sts):
                          if MODE == "qk_only":
                              continue
                          pt = ptp.tile([128, 1024], bf, tag="pt",
                                        name=f"pt_{rep}_{h}_{kb}_{hs}")
                          nc.scalar.activation(pt[:, :hw], st[:, :hw], Exp,
                                               scale=0.125)
                          if causal and hs == 0:
                              nc.vector.tensor_mul(pt[:, :128], pt[:, :128],
                                                   tri_t[:, :])
                          pts.append(pt)
                      # QK for strip kb+1 goes to PE before PV of strip kb so
                      # the PE never stalls behind ACT
                      if kb + 1 < KB:
                          sts = emit_qk(kb + 1)
                      # PV accumulation for strip kb
                      if MODE != "full":
                          continue
                      for (q0, hs, hw, tg), pt in zip(strip_halves(kb), pts):
                          qs_range = list(range((q0 + hs) // 128,
                                                (q0 + hs + hw) // 128))
                          # the diagonal q_sub (== kb) additionally depends on
                          # the DVE tri-multiply; emit it last so the PE can
                          # start the other PV matmuls as soon as exp is done.
                          # (at kb==0 keep ascending order: the bank-group
                          # start=True matmuls must be first into each bank)
                          if causal and kb > 0 and qs_range and qs_range[0] == kb:
                              qs_range = qs_range[1:] + [kb]
                          for q_sub in qs_range:
                              m = q_sub * 128 - q0 - hs
                              last_kb = q_sub if causal else KB - 1
                              nc.tensor.matmul(
                                  acc(q_sub),
                                  lhsT=pt[:, m:m + 128],
                                  rhs=vg_t[:, kb, :],
                                  start=(kb == 0 and q_sub in _BANK_FIRST),
                                  stop=(q_sub in _BANK_LAST and kb == last_kb),
                              )
                      # normalize accumulator banks as soon as they complete
                      if causal:
                          if kb == 6:
                              normalize_bank(0, 7)
                          elif kb == 13:
                              normalize_bank(7, 14)
                          elif kb == 15:
                              normalize_bank(14, 16)
                  if not causal:
                      normalize_bank(0, 7)
                      normalize_bank(7, 14)
                      normalize_bank(14, 16)


def build_nc(causal=True, reps=1):
    """Build + compile the per-core Bass program (cached)."""
    key = ("nc", causal, reps, STRIP_W, MODE, ST_BUFS)
    if key in _built:
        return _built[key]
    import concourse.bacc as bacc
    from concourse import mybir, tile

    nc = bacc.Bacc("TRN2", target_bir_lowering=False, debug=False,
                   num_devices=N_CORES)
    qt = nc.dram_tensor("qt", (HEADS_PER_CORE // 2, 128, S),
                        mybir.dt.bfloat16, kind="ExternalInput").ap()
    kt = nc.dram_tensor("kt", (HEADS_PER_CORE // 2, 128, S),
                        mybir.dt.bfloat16, kind="ExternalInput").ap()
    vg = nc.dram_tensor("vg", (HEADS_PER_CORE, 128, KB, 65),
                        mybir.dt.bfloat16, kind="ExternalInput").ap()
    tri = nc.dram_tensor("tri", (128, 128), mybir.dt.bfloat16,
                         kind="ExternalInput").ap()
    o = nc.dram_tensor("o", (HEADS_PER_CORE, S, D), mybir.dt.float32,
                       kind="ExternalOutput").ap()
    with tile.TileContext(nc) as tc:
        _emit(tc, nc, mybir, qt, kt, vg, tri, o, causal, reps)
    nc.compile()
    _built[key] = nc
    return nc


def prep_inputs(Q, K, V):
    """Host-side shard + layout prep. Returns list of 8 per-core input dicts."""
    Qf = np.ascontiguousarray(Q, dtype=np.float32).reshape(B * H, S, D)
    Kf = np.ascontiguousarray(K, dtype=np.float32).reshape(B * H, S, D)
    Vf = np.ascontiguousarray(V, dtype=np.float32).reshape(B * H, S, D)

    # [BH, S, D] -> transposed, bf16: [BH, D, S]
    Qt = np.ascontiguousarray(Qf.transpose(0, 2, 1)).astype(_BF16)
    Kt = np.ascontiguousarray(Kf.transpose(0, 2, 1)).astype(_BF16)

    # V augmented with ones column, partition-major: [BH, 128, KB, 65]
    Vb = Vf.astype(_BF16)
    vg_all = np.empty((B * H, 128, KB, 65), dtype=_BF16)
    # V[h, kb*128 + r, c] -> vg[h, r, kb, c]
    vg_all[:, :, :, :64] = Vb.reshape(B * H, KB, 128, D).transpose(0, 2, 1, 3)
    vg_all[:, :, :, 64] = _BF16(1.0)

    tri_np = (np.tril(np.ones((128, 128), dtype=np.float32))
              .T.astype(_BF16))  # tri[k, c] = 1 if c >= k
    tri_np = np.ascontiguousarray(tri_np)

    in_maps = []
    for c in range(N_CORES):
        h0 = c * HEADS_PER_CORE
        qt_c = np.empty((HEADS_PER_CORE // 2, 128, S), dtype=_BF16)
        kt_c = np.empty((HEADS_PER_CORE // 2, 128, S), dtype=_BF16)
        for p in range(HEADS_PER_CORE // 2):
            qt_c[p, :64] = Qt[h0 + 2 * p]
            qt_c[p, 64:] = Qt[h0 + 2 * p + 1]
            kt_c[p, :64] = Kt[h0 + 2 * p]
            kt_c[p, 64:] = Kt[h0 + 2 * p + 1]
        in_maps.append({
            "qt": qt_c,
            "kt": kt_c,
            "vg": np.ascontiguousarray(vg_all[h0:h0 + HEADS_PER_CORE]),
            "tri": tri_np,
        })
    return in_maps


def _classify_mask(mask):
    m = np.asarray(mask).reshape(S, S)
    if not m.any():
        return "dense"
    if np.array_equal(m, np.triu(np.ones((S, S), dtype=bool), k=1)):
        return "causal"
    raise NotImplementedError("only causal or all-False masks supported")


def run_cores(in_maps, causal=True, reps=1, **kwargs):
    from concourse import bass_utils

    nc = build_nc(causal, reps)
    return bass_utils.run_bass_kernel_spmd(
        nc, in_maps, core_ids=list(range(N_CORES)), **kwargs
    )


def kernel(Q, K, V, mask):
    kind = _classify_mask(mask)
    in_maps = prep_inputs(Q, K, V)
    res = run_cores(in_maps, causal=(kind == "causal"))
    out = np.concatenate([r["o"] for r in res.results], axis=0)
    return out.reshape(B, H, S, D).astype(np.float32)


if __name__ == "__main__":
    rng = np.random.default_rng(0)
    Q = rng.standard_normal((B, H, S, D), dtype=np.float32)
    K = rng.standard_normal((B, H, S, D), dtype=np.float32)
    V = rng.standard_normal((B, H, S, D), dtype=np.float32)
    mask = np.triu(np.ones((S, S), dtype=bool), k=1)[None, None]
    out = kernel(Q, K, V, mask)
    print("out", out.shape, out.dtype)

